# revision 24
# baseline (speedup 1.0000x reference)
"""Trainium2 Bass kernel for nn_AveragedAdapter (dense_mlp).

Computes: loss = sum_{a,e} mean_{b,d} (gelu(f[:,a] @ W1[a,e] + b1[a,e]) @ W2[a,e]
                                        + b2[a,e] - target[:,a])^2 / E

Sharding: expert-parallel over the first expert axis `a` — core a computes the
full inner-e loop for its adapter row and returns per-partition partial sums of
squared errors; the host adds the 8x[128,8] partials and applies the
1/(B*D*E) scale.

The 512 MiB of weights dominate the roofline (each element used exactly once),
so weights (plus features and the hidden activations) are carried in fp8-e4m3:
the final scalar is a mean over 33.5M squared errors and is insensitive to
weight rounding.  Biases, targets and all accumulation stay >= bf16 (matmuls
accumulate in fp32 PSUM).

A pure-DMA probe showed the per-core HBM->SBUF stream sustains ~380 GB/s in
isolation but only ~320 GB/s inside this kernel — the difference is SBUF
port/bank contention from concurrent engine traffic. The kernel therefore
minimizes non-essential SBUF traffic while the weight stream is in flight:

  - phase 1 (all experts): layer 1 computes hT (H on partitions) with W1
    chunk-pairs stationary via fp8 DoubleRow ([128,2,128] x [128,2,128]),
    4 m-chunks per PSUM bank. The bias add runs on DVE PSUM -> PSUM
    (broadcast over batch), then one ACT pass per group applies exact-erf
    Gelu PSUM -> fp8 h in SBUF. No intermediate z tile in SBUF (saves
    ~8 MB of SBUF round-trip traffic that used to slow the weight
    stream; per-m-chunk ACT-bias gelus were tried instead and lose —
    ACT's ~300ns/instruction overhead makes 4 small gelus per group far
    slower than one big one).
  - phase 2 (all experts): PSUM is preloaded with (b2[a,e] - target[:,a])
    — a rank-1 ones x b2[a,e] matmul initializes the bank, then DVE
    (idle in phase 2) adds -target in place — and the 8 fp8 DoubleRow
    layer-2 matmuls accumulate h @ W2 on top, so the PSUM bank ends up
    holding err directly. ACT squares it (reading PSUM) and row-sums into column
    e of one [128, E] fp32 tile (accum_out). This removes the per-expert
    [128,512] target tensors (-0.9 MB of input stream) and the DVE
    subtract + err round-trip (-4 MB of SBUF traffic), and shortens the
    kernel tail to Square -> one output DMA.
  - All weight slabs ride the sync HWDGE ring in consumption order (W1
    slabs, then W2 slabs, the last two W2 slabs in k-halves); probes
    showed slab size doesn't change throughput, the gpsimd SWDGE queue
    is slower, and a sync/scalar split jams the scalar sequencer.
    ft/b1 lead the sync ring (phase-1 start), tg/b2 crawl on the scalar
    ring (needed late). Throwaway matmuls warm the PE clock-gate
    (1.2 -> 2.4 GHz) during the first slab's flight.
"""

import sys

if "/opt/trn_rl_repo" not in sys.path:
    sys.path.insert(0, "/opt/trn_rl_repo")

import numpy as np
import ml_dtypes

B, E, D, M = 128, 8, 512, 4
H = M * D            # 2048
P = 128
KC1 = D // P         # 4  k-chunks in layer 1
MC = H // P          # 16 m-chunks of H / k-chunks in layer 2
NG = 4               # m-chunk groups (4 chunks -> one PSUM bank)
W1_COLS = KC1 * H    # 8192
W2_COLS = MC * D     # 8192
F8 = ml_dtypes.float8_e4m3
BF16 = ml_dtypes.bfloat16

_NC = None


def _build_nc(act="gelu"):
    import concourse.tile as tile
    from concourse import bacc, mybir

    act_fn = {
        "gelu": mybir.ActivationFunctionType.Gelu,
        "identity": mybir.ActivationFunctionType.Identity,
    }[act]
    # Bacc (not Bass): its compile() pass legalizes sync waits for the trn2
    # ISA's one-wait-per-instruction limit (move_matmul_waits_to_ldweights +
    # generate_event_semaphores) — walrus codegen rejects multi-wait
    # instructions otherwise.
    nc = bacc.Bacc(None)
    f8 = mybir.dt.float8e4
    f32 = mybir.dt.float32

    bf16 = mybir.dt.bfloat16
    wpack = nc.dram_tensor("wpack", [E, P, W1_COLS + W2_COLS], f8, kind="ExternalInput")
    ftp = nc.dram_tensor("ftp", [P, KC1, B], f8, kind="ExternalInput")
    tgtn = nc.dram_tensor("tgtn", [P, D], bf16, kind="ExternalInput")
    b2p = nc.dram_tensor("b2p", [1, E, D], bf16, kind="ExternalInput")
    b1p = nc.dram_tensor("b1p", [P, E, MC], f32, kind="ExternalInput")
    loss = nc.dram_tensor("loss", [P, E], f32, kind="ExternalOutput")

    with tile.TileContext(nc) as tc:
        with (
            tc.tile_pool(name="w1pool", bufs=E) as w1pool,
            tc.tile_pool(name="w2pool", bufs=E) as w2pool,
            tc.tile_pool(name="cpool", bufs=1) as cpool,
            tc.tile_pool(name="hpool", bufs=E) as hpool,
            tc.tile_pool(name="epool", bufs=3) as epool,
            tc.tile_pool(name="psz", bufs=3, space="PSUM") as psz,
            tc.tile_pool(name="psb", bufs=2, space="PSUM") as psb,
            tc.tile_pool(name="pso", bufs=3, space="PSUM") as pso,
        ):
            # Phase-1's inputs (ft, b1 — 0.13 MB) lead the SYNC ring: on the
            # scalar ring their small packets crawl behind the weight
            # stream's 8-KB packets (packet-granular round-robin) and the
            # phase-1 pipeline start slips ~5us, which puts PE — the tail
            # critical path — 5us later. Leading sync costs the stream only
            # ~0.4us. Phase-2's inputs (tg, b2) can crawl on the scalar ring
            # (needed only at ~40us).
            ft = cpool.tile([P, KC1, B], f8)
            nc.sync.dma_start(ft[:], ftp[:])
            b1s = cpool.tile([P, E, MC], f32)
            nc.sync.dma_start(b1s[:], b1p[:])
            tg = cpool.tile([P, D], bf16)
            nc.scalar.dma_start(tg[:], tgtn[:])
            b2s = cpool.tile([1, E, D], bf16)
            nc.scalar.dma_start(b2s[:], b2p[:])
            wsrc = cpool.tile([P, D], f8)
            nc.vector.memset(wsrc[:], 0.0)
            ones1 = cpool.tile([1, P], bf16)
            nc.vector.memset(ones1[:], 1.0)
            # Advance the DVE vector clock past the b1s DMA with a one-element
            # read so the bias-add TTs only need their PE wait (trn2 allows
            # one wait per instruction).
            dummy = cpool.tile([1, 4], mybir.dt.float32)
            nc.vector.tensor_copy(dummy[:, 0:1], b1s[:1, 0, :1])

            # Warm the PE HAM clock-gate (idle PE runs at 1.2 GHz; ~3.4us of
            # sustained activity unlocks 2.4 GHz) with throwaway matmuls on a
            # zeroed tile while the first weight slab is still in flight.
            # 10 x ~380ns = ~3.8us of activity — enough to unlock the clock,
            # ending right as W1[0] lands (the PE executes its queue in
            # order, so an over-long warmup pushes all of phase 1 back).
            pwarm = pso.tile([P, D], mybir.dt.float32, tag="po")
            NWARM = 11
            for i in range(NWARM):
                nc.tensor.matmul(
                    pwarm[:], lhsT=wsrc[:, :P], rhs=wsrc[:],
                    start=(i == 0), stop=(i == NWARM - 1),
                )

            # Weight slab delivery: ALL slabs on the sync HWDGE ring — the
            # sync sequencer does nothing else, so issues never wait behind
            # compute. (A sync/scalar 8-8 split was tried and loses badly:
            # the scalar sequencer serializes its DMA issues — which carry
            # completion-semaphore waits — with the gelu ACTIVATEs, pushing
            # the first gelu to 45us.) The ring is FIFO, so this IS the
            # arrival order: W1 slabs ahead of W2 slabs, matching the
            # two-phase consumption order. The last two W2 slabs are issued
            # in two k-chunk halves each so the final experts' first four
            # layer-2 matmuls start while their second halves are still
            # landing.
            w1ts, w2ts = {}, {}

            def issue_w1(e):
                w1ts[e] = w1pool.tile([P, W1_COLS], f8, tag="w1", name=f"w1t{e}")
                nc.sync.dma_start(w1ts[e][:], wpack[e][:, :W1_COLS])

            def issue_w2(e, split=False):
                w2ts[e] = w2pool.tile([P, MC // 2, 2, D], f8, tag="w2", name=f"w2t{e}")
                src = wpack[e][:, W1_COLS:].rearrange(
                    "p (k two d) -> p k two d", two=2, d=D
                )
                if split:
                    half = MC // 4
                    nc.sync.dma_start(w2ts[e][:, :half], src[:, :half])
                    nc.sync.dma_start(w2ts[e][:, half:], src[:, half:])
                else:
                    nc.sync.dma_start(w2ts[e][:], src)

            # Ring order exploits phase-1's arrival slack (W1 slabs land
            # ~2.55us apart but L1 consumes ~3.35us/expert, building ~5us of
            # slack): W2[0] and W2[1] are pulled forward between W1 slabs so
            # phase-2 never stalls on its first arrivals, and W1[0] is issued
            # in two k-chunk halves so expert 0's kc=0 matmuls start one
            # half-slab earlier. The last two W2 slabs are split for the same
            # reason at the tail.
            w1ts[0] = w1pool.tile([P, W1_COLS], f8, tag="w1", name="w1t0")
            nc.sync.dma_start(w1ts[0][:, : W1_COLS // 2], wpack[0][:, : W1_COLS // 2])
            nc.sync.dma_start(
                w1ts[0][:, W1_COLS // 2 :], wpack[0][:, W1_COLS // 2 : W1_COLS]
            )
            for e in (1, 2, 3):
                issue_w1(e)
            issue_w2(0)
            for e in (4, 5, 6):
                issue_w1(e)
            issue_w2(1)
            issue_w1(7)
            for e in range(2, E):
                issue_w2(e, split=(e >= E - 2))

            # Phase 1: layer-1 + gelu for ALL experts. PE executes its queue
            # in program order, so trailing layer-2 work must not sit between
            # layer-1 passes — this way the last expert's bias/gelu chain
            # drains while layer-2 matmuls for earlier experts run, instead of
            # serializing at the end of the kernel.
            hsbs = {}
            for e in range(E):
                w1v = w1ts[e][:].rearrange("p (k h) -> p k h", k=KC1)
                hsb = hpool.tile([P, MC, P], f8, tag="h", name=f"hsb{e}")
                hsbs[e] = hsb
                for g in range(NG):
                    zp = psz.tile([P, NG, P], mybir.dt.float32, tag="zp")
                    for mc in range(NG):
                        m = g * NG + mc
                        # fp8 DoubleRow on layer 1 as well: contract two
                        # 128-row D-chunks per matmul (half the instruction
                        # count; the exposed per-matmul LDWEIGHTS cost is what
                        # limits layer 1, since N=B=128 is short).
                        for kc in range(KC1 // 2):
                            nc.tensor.matmul(
                                zp[:, mc],
                                lhsT=w1v[:, 2 * kc : 2 * kc + 2, m * P : (m + 1) * P],
                                rhs=ft[:, 2 * kc : 2 * kc + 2, :],
                                start=(kc == 0),
                                stop=(kc == KC1 // 2 - 1),
                                perf_mode=mybir.MatmulPerfMode.DoubleRow,
                            )
                    # bias add on DVE, PSUM -> PSUM (no SBUF round-trip),
                    # then one big gelu on ACT, PSUM -> fp8 h in SBUF.
                    zb = psb.tile([P, NG, P], mybir.dt.float32, tag="zb")
                    nc.vector.tensor_tensor(
                        zb[:],
                        zp[:],
                        b1s[:, e, g * NG : (g + 1) * NG, None].to_broadcast([P, NG, P]),
                        mybir.AluOpType.add,
                    )
                    nc.scalar.activation(
                        hsb[:, g * NG : (g + 1) * NG],
                        zb[:],
                        act_fn,
                    )

            # Phase 2: layer-2 + per-expert loss accumulation. PSUM starts as
            # (b2[e] - target) via two cheap bf16 matmuls (identity x (-tgt),
            # ones x b2[e]); the 8 fp8 DoubleRow matmuls accumulate h @ W2 on
            # top, leaving err in PSUM. ACT squares it and row-sums into
            # column e of one [128, E] fp32 tile; a single DMA ships the tile
            # after the last expert and the host does the final sum.
            red_all = cpool.tile([P, E], mybir.dt.float32)
            pos = {}

            def preload(e):
                # preload PSUM with (b2[e] - target): a rank-1 ones x b2
                # matmul initializes the bank (start=True), then DVE (idle in
                # phase 2) adds -target in place; the layer-2 matmuls
                # accumulate on top. The chain is strictly PE -> DVE -> PE so
                # every instruction needs a single wait (a CAST-then-matmul
                # variant raced on hardware). Preloads run two experts ahead
                # (pso bufs=3) so this never sits on the per-expert critical
                # path — and the rank-1 lands between expert DR blocks,
                # keeping PE busy enough that the HAM clock-gate never drops
                # it back to 1.2 GHz.
                pos[e] = pso.tile([P, D], mybir.dt.float32, tag="po", name=f"po{e}")
                nc.tensor.matmul(
                    pos[e][:], lhsT=ones1[:], rhs=b2s[:, e], start=True,
                    stop=False, skip_group_check=True,
                )
                nc.vector.tensor_tensor(
                    pos[e][:], pos[e][:], tg[:], mybir.AluOpType.add
                )

            # advance the DVE clock past the tg DMA (same single-wait trick
            # as b1s; tg rides the scalar ring and lands ~25us, well before
            # this point in the DVE queue)
            nc.vector.tensor_copy(dummy[:, 1:2], tg[:1, :1])
            preload(0)
            preload(1)
            for e in range(E):
                w2t, hsb, po = w2ts[e], hsbs[e], pos[e]
                # fp8 DoubleRow: each matmul contracts a pair of 128-row
                # k-chunks (array virtualized to 256 rows) — halves layer-2's
                # PE cycles. lhsT [128,2,128] = adjacent h chunks; rhs
                # [128,2,512] = the matching W2 chunk pair.
                for kc in range(MC // 2):
                    nc.tensor.matmul(
                        po[:],
                        lhsT=hsb[:, 2 * kc : 2 * kc + 2, :],
                        rhs=w2t[:, kc],
                        start=False,
                        stop=(kc == MC // 2 - 1),
                        perf_mode=mybir.MatmulPerfMode.DoubleRow,
                        skip_group_check=True,
                    )
                if e + 2 < E:
                    preload(e + 2)

                # square + row-sum in one ACT pass reading PSUM (fp32
                # accumulator); the Square output itself is scrap (fp8 to
                # minimize SBUF write traffic)
                sq = epool.tile([P, D], f8, tag="sq")
                nc.scalar.activation(
                    sq[:], po[:], mybir.ActivationFunctionType.Square,
                    accum_out=red_all[:, e : e + 1],
                )

            nc.sync.dma_start(loss[:], red_all[:])

    nc.finalize()
    return nc


def get_nc(act="gelu"):
    global _NC
    if _NC is None:
        _NC = _build_nc(act)
    return _NC


def make_in_maps(features, target_features, W1, b1, W2, b2):
    features = np.asarray(features, np.float32)
    target_features = np.asarray(target_features, np.float32)
    W1 = np.asarray(W1, np.float32)
    b1 = np.asarray(b1, np.float32)
    W2 = np.asarray(W2, np.float32)
    b2 = np.asarray(b2, np.float32)

    # pack weights partition-major: wpack[a][e][p, kc*H + col] = W1[a,e,kc*128+p,col]
    #                              wpack[a][e][p, 8192 + kc*D + d] = W2[a,e,kc*128+p,d]
    w1p = np.ascontiguousarray(
        W1.reshape(E, E, KC1, P, H).transpose(0, 1, 3, 2, 4).reshape(E, E, P, W1_COLS)
    ).astype(F8)
    w2p = np.ascontiguousarray(
        W2.reshape(E, E, MC, P, D).transpose(0, 1, 3, 2, 4).reshape(E, E, P, W2_COLS)
    ).astype(F8)
    wpk = np.concatenate([w1p, w2p], axis=3)  # [A, E, P, 16384] fp8

    in_maps = []
    for a in range(E):
        fa = features[:, a]  # [B, D]
        ftp = np.ascontiguousarray(fa.T.reshape(KC1, P, B).transpose(1, 0, 2)).astype(F8)
        tgtn = np.ascontiguousarray(-target_features[:, a]).astype(BF16)  # [B, D]
        b2pa = np.ascontiguousarray(b2[a][None]).astype(BF16)  # [1, E, D]
        b1pa = np.ascontiguousarray(b1[a].reshape(E, MC, P).transpose(2, 0, 1))  # [P,E,MC]
        in_maps.append(
            {"wpack": wpk[a], "ftp": ftp, "tgtn": tgtn, "b2p": b2pa,
             "b1p": b1pa}
        )
    return in_maps


def kernel(features, target_features, W1, b1, W2, b2):
    from concourse.bass_utils import run_bass_kernel_spmd

    nc = get_nc()
    in_maps = make_in_maps(features, target_features, W1, b1, W2, b2)
    res = run_bass_kernel_spmd(nc, in_maps, list(range(E)))
    total = sum(float(r["loss"].astype(np.float64).sum()) for r in res.results)
    return np.float32(total / (B * D * E))


# revision 25
# speedup vs baseline: 1.1288x; 1.1288x over previous
"""Trainium2 Bass kernel for nn_AveragedAdapter (dense_mlp).

Computes: loss = sum_{a,e} mean_{b,d} (gelu(f[:,a] @ W1[a,e] + b1[a,e]) @ W2[a,e]
                                        + b2[a,e] - target[:,a])^2 / E

Sharding: expert-parallel over the first expert axis `a` — core a computes the
full inner-e loop for its adapter row and returns per-partition partial sums of
squared errors; the host adds the 8x[128,8] partials and applies the
1/(B*D*E) scale.

The 512 MiB of weights dominate the roofline (each element used exactly once),
so weights (plus features and the hidden activations) are carried in fp8-e4m3:
the final scalar is a mean over 33.5M squared errors and is insensitive to
weight rounding.  Biases, targets and all accumulation stay >= bf16 (matmuls
accumulate in fp32 PSUM).

A pure-DMA probe showed the per-core HBM->SBUF stream sustains ~380 GB/s in
isolation but only ~320 GB/s inside this kernel — the difference is SBUF
port/bank contention from concurrent engine traffic. The kernel therefore
minimizes non-essential SBUF traffic while the weight stream is in flight:

  - phase 1 (all experts): layer 1 computes hT (H on partitions) with W1
    chunk-pairs stationary via fp8 DoubleRow ([128,2,128] x [128,2,128]),
    4 m-chunks per PSUM bank. The bias add runs on DVE PSUM -> PSUM
    (broadcast over batch), then one ACT pass per group applies exact-erf
    Gelu PSUM -> fp8 h in SBUF. No intermediate z tile in SBUF (saves
    ~8 MB of SBUF round-trip traffic that used to slow the weight
    stream; per-m-chunk ACT-bias gelus were tried instead and lose —
    ACT's ~300ns/instruction overhead makes 4 small gelus per group far
    slower than one big one).
  - phase 2 (all experts): PSUM is preloaded with (b2[a,e] - target[:,a])
    — a rank-1 ones x b2[a,e] matmul initializes the bank, then DVE
    (idle in phase 2) adds -target in place — and the 8 fp8 DoubleRow
    layer-2 matmuls accumulate h @ W2 on top, so the PSUM bank ends up
    holding err directly. ACT squares it (reading PSUM) and row-sums into column
    e of one [128, E] fp32 tile (accum_out). This removes the per-expert
    [128,512] target tensors (-0.9 MB of input stream) and the DVE
    subtract + err round-trip (-4 MB of SBUF traffic), and shortens the
    kernel tail to Square -> one output DMA.
  - All weight slabs ride the sync HWDGE ring in consumption order (W1
    slabs, then W2 slabs, the last two W2 slabs in k-halves); probes
    showed slab size doesn't change throughput, the gpsimd SWDGE queue
    is slower, and a sync/scalar split jams the scalar sequencer.
    ft/b1 lead the sync ring (phase-1 start), tg/b2 crawl on the scalar
    ring (needed late). Throwaway matmuls warm the PE clock-gate
    (1.2 -> 2.4 GHz) during the first slab's flight.
"""

import sys

if "/opt/trn_rl_repo" not in sys.path:
    sys.path.insert(0, "/opt/trn_rl_repo")

import numpy as np
import ml_dtypes

B, E, D, M = 128, 8, 512, 4
H = M * D            # 2048
P = 128
KC1 = D // P         # 4  k-chunks in layer 1
MC = H // P          # 16 m-chunks of H / k-chunks in layer 2
NG = 4               # m-chunk groups (4 chunks -> one PSUM bank)
W1_COLS = KC1 * H    # 8192
W2_COLS = MC * D     # 8192
F8 = ml_dtypes.float8_e4m3
BF16 = ml_dtypes.bfloat16

_NC = None


def _build_nc(act="gelu"):
    import concourse.tile as tile
    from concourse import bacc, mybir

    act_fn = {
        "gelu": mybir.ActivationFunctionType.Gelu,
        "identity": mybir.ActivationFunctionType.Identity,
    }[act]
    # Bacc (not Bass): its compile() pass legalizes sync waits for the trn2
    # ISA's one-wait-per-instruction limit (move_matmul_waits_to_ldweights +
    # generate_event_semaphores) — walrus codegen rejects multi-wait
    # instructions otherwise.
    nc = bacc.Bacc(None)
    f8 = mybir.dt.float8e4
    f32 = mybir.dt.float32

    bf16 = mybir.dt.bfloat16
    wpack = nc.dram_tensor("wpack", [E, P, W1_COLS + W2_COLS], f8, kind="ExternalInput")
    ftp = nc.dram_tensor("ftp", [P, KC1, B], f8, kind="ExternalInput")
    tgtn = nc.dram_tensor("tgtn", [P, D], bf16, kind="ExternalInput")
    b2p = nc.dram_tensor("b2p", [1, E, D], bf16, kind="ExternalInput")
    b1p = nc.dram_tensor("b1p", [P, E, MC], f32, kind="ExternalInput")
    loss = nc.dram_tensor("loss", [P, E], f32, kind="ExternalOutput")

    with tile.TileContext(nc) as tc:
        with (
            tc.tile_pool(name="w1pool", bufs=E) as w1pool,
            tc.tile_pool(name="w2pool", bufs=E) as w2pool,
            tc.tile_pool(name="cpool", bufs=1) as cpool,
            tc.tile_pool(name="hpool", bufs=E) as hpool,
            tc.tile_pool(name="epool", bufs=3) as epool,
            tc.tile_pool(name="psz", bufs=3, space="PSUM") as psz,
            tc.tile_pool(name="psb", bufs=2, space="PSUM") as psb,
            tc.tile_pool(name="pso", bufs=3, space="PSUM") as pso,
        ):
            # Phase-1's inputs (ft, b1 — 0.13 MB) lead the SYNC ring: on the
            # scalar ring their small packets crawl behind the weight
            # stream's 8-KB packets (packet-granular round-robin) and the
            # phase-1 pipeline start slips ~5us, which puts PE — the tail
            # critical path — 5us later. Leading sync costs the stream only
            # ~0.4us. Phase-2's inputs (tg, b2) can crawl on the scalar ring
            # (needed only at ~40us).
            ft = cpool.tile([P, KC1, B], f8)
            nc.sync.dma_start(ft[:], ftp[:])
            b1s = cpool.tile([P, E, MC], f32)
            nc.sync.dma_start(b1s[:], b1p[:])
            tg = cpool.tile([P, D], bf16)
            nc.scalar.dma_start(tg[:], tgtn[:])
            b2s = cpool.tile([1, E, D], bf16)
            nc.scalar.dma_start(b2s[:], b2p[:])
            wsrc = cpool.tile([P, D], f8)
            nc.vector.memset(wsrc[:], 0.0)
            ones1 = cpool.tile([1, P], bf16)
            nc.vector.memset(ones1[:], 1.0)
            # Advance the DVE vector clock past the b1s DMA with a one-element
            # read so the bias-add TTs only need their PE wait (trn2 allows
            # one wait per instruction).
            dummy = cpool.tile([1, 4], mybir.dt.float32)
            nc.vector.tensor_copy(dummy[:, 0:1], b1s[:1, 0, :1])

            # Warm the PE HAM clock-gate (idle PE runs at 1.2 GHz; ~3.4us of
            # sustained activity unlocks 2.4 GHz) with throwaway matmuls on a
            # zeroed tile while the first weight slab is still in flight.
            # 10 x ~380ns = ~3.8us of activity — enough to unlock the clock,
            # ending right as W1[0] lands (the PE executes its queue in
            # order, so an over-long warmup pushes all of phase 1 back).
            pwarm = pso.tile([P, D], mybir.dt.float32, tag="po")
            NWARM = 11
            for i in range(NWARM):
                nc.tensor.matmul(
                    pwarm[:], lhsT=wsrc[:, :P], rhs=wsrc[:],
                    start=(i == 0), stop=(i == NWARM - 1),
                )

            # Weight slab delivery: ALL slabs on the sync HWDGE ring — the
            # sync sequencer does nothing else, so issues never wait behind
            # compute. (A sync/scalar 8-8 split was tried and loses badly:
            # the scalar sequencer serializes its DMA issues — which carry
            # completion-semaphore waits — with the gelu ACTIVATEs, pushing
            # the first gelu to 45us.) The ring is FIFO, so this IS the
            # arrival order: W1 slabs ahead of W2 slabs, matching the
            # two-phase consumption order. The last two W2 slabs are issued
            # in two k-chunk halves each so the final experts' first four
            # layer-2 matmuls start while their second halves are still
            # landing.
            w1ts, w2ts = {}, {}

            def issue_w1(e):
                w1ts[e] = w1pool.tile([P, W1_COLS], f8, tag="w1", name=f"w1t{e}")
                nc.sync.dma_start(w1ts[e][:], wpack[e][:, :W1_COLS])

            def issue_w2(e, split=False):
                w2ts[e] = w2pool.tile([P, MC // 2, 2, D], f8, tag="w2", name=f"w2t{e}")
                src = wpack[e][:, W1_COLS:].rearrange(
                    "p (k two d) -> p k two d", two=2, d=D
                )
                if split:
                    half = MC // 4
                    nc.sync.dma_start(w2ts[e][:, :half], src[:, :half])
                    nc.sync.dma_start(w2ts[e][:, half:], src[:, half:])
                else:
                    nc.sync.dma_start(w2ts[e][:], src)

            # Ring order exploits phase-1's arrival slack (W1 slabs land
            # ~2.55us apart but L1 consumes ~3.35us/expert, building ~5us of
            # slack): W2[0] and W2[1] are pulled forward between W1 slabs so
            # phase-2 never stalls on its first arrivals, and W1[0] is issued
            # in two k-chunk halves so expert 0's kc=0 matmuls start one
            # half-slab earlier. The last two W2 slabs are split for the same
            # reason at the tail.
            for e in (0, 1, 2, 3):
                issue_w1(e)
            issue_w2(0)
            for e in (4, 5, 6):
                issue_w1(e)
            issue_w2(1)
            issue_w1(7)
            for e in range(2, E):
                issue_w2(e, split=(e >= E - 2))

            # Phase 1: layer-1 + gelu for ALL experts. PE executes its queue
            # in program order, so trailing layer-2 work must not sit between
            # layer-1 passes — this way the last expert's bias/gelu chain
            # drains while layer-2 matmuls for earlier experts run, instead of
            # serializing at the end of the kernel.
            hsbs = {}
            for e in range(E):
                w1v = w1ts[e][:].rearrange("p (k h) -> p k h", k=KC1)
                hsb = hpool.tile([P, MC, P], f8, tag="h", name=f"hsb{e}")
                hsbs[e] = hsb
                for g in range(NG):
                    zp = psz.tile([P, NG, P], mybir.dt.float32, tag="zp")
                    for mc in range(NG):
                        m = g * NG + mc
                        # fp8 DoubleRow on layer 1 as well: contract two
                        # 128-row D-chunks per matmul (half the instruction
                        # count; the exposed per-matmul LDWEIGHTS cost is what
                        # limits layer 1, since N=B=128 is short).
                        for kc in range(KC1 // 2):
                            nc.tensor.matmul(
                                zp[:, mc],
                                lhsT=w1v[:, 2 * kc : 2 * kc + 2, m * P : (m + 1) * P],
                                rhs=ft[:, 2 * kc : 2 * kc + 2, :],
                                start=(kc == 0),
                                stop=(kc == KC1 // 2 - 1),
                                perf_mode=mybir.MatmulPerfMode.DoubleRow,
                            )
                    # bias add on DVE, PSUM -> PSUM (no SBUF round-trip),
                    # then one big gelu on ACT, PSUM -> fp8 h in SBUF.
                    zb = psb.tile([P, NG, P], mybir.dt.float32, tag="zb")
                    nc.vector.tensor_tensor(
                        zb[:],
                        zp[:],
                        b1s[:, e, g * NG : (g + 1) * NG, None].to_broadcast([P, NG, P]),
                        mybir.AluOpType.add,
                    )
                    nc.scalar.activation(
                        hsb[:, g * NG : (g + 1) * NG],
                        zb[:],
                        act_fn,
                    )

            # Phase 2: layer-2 + per-expert loss accumulation. PSUM starts as
            # (b2[e] - target) via two cheap bf16 matmuls (identity x (-tgt),
            # ones x b2[e]); the 8 fp8 DoubleRow matmuls accumulate h @ W2 on
            # top, leaving err in PSUM. ACT squares it and row-sums into
            # column e of one [128, E] fp32 tile; a single DMA ships the tile
            # after the last expert and the host does the final sum.
            red_all = cpool.tile([P, E], mybir.dt.float32)
            pos = {}

            def preload(e):
                # preload PSUM with (b2[e] - target): a rank-1 ones x b2
                # matmul initializes the bank (start=True), then DVE (idle in
                # phase 2) adds -target in place; the layer-2 matmuls
                # accumulate on top. The chain is strictly PE -> DVE -> PE so
                # every instruction needs a single wait (a CAST-then-matmul
                # variant raced on hardware). Preloads run two experts ahead
                # (pso bufs=3) so this never sits on the per-expert critical
                # path — and the rank-1 lands between expert DR blocks,
                # keeping PE busy enough that the HAM clock-gate never drops
                # it back to 1.2 GHz.
                pos[e] = pso.tile([P, D], mybir.dt.float32, tag="po", name=f"po{e}")
                nc.tensor.matmul(
                    pos[e][:], lhsT=ones1[:], rhs=b2s[:, e], start=True,
                    stop=False, skip_group_check=True,
                )
                nc.vector.tensor_tensor(
                    pos[e][:], pos[e][:], tg[:], mybir.AluOpType.add
                )

            # advance the DVE clock past the tg DMA (same single-wait trick
            # as b1s; tg rides the scalar ring and lands ~25us, well before
            # this point in the DVE queue)
            nc.vector.tensor_copy(dummy[:, 1:2], tg[:1, :1])
            preload(0)
            preload(1)
            for e in range(E):
                w2t, hsb, po = w2ts[e], hsbs[e], pos[e]
                # fp8 DoubleRow: each matmul contracts a pair of 128-row
                # k-chunks (array virtualized to 256 rows) — halves layer-2's
                # PE cycles. lhsT [128,2,128] = adjacent h chunks; rhs
                # [128,2,512] = the matching W2 chunk pair.
                for kc in range(MC // 2):
                    nc.tensor.matmul(
                        po[:],
                        lhsT=hsb[:, 2 * kc : 2 * kc + 2, :],
                        rhs=w2t[:, kc],
                        start=False,
                        stop=(kc == MC // 2 - 1),
                        perf_mode=mybir.MatmulPerfMode.DoubleRow,
                        skip_group_check=True,
                    )
                if e + 2 < E:
                    preload(e + 2)

                # square + row-sum in one ACT pass reading PSUM (fp32
                # accumulator); the Square output itself is scrap (fp8 to
                # minimize SBUF write traffic)
                sq = epool.tile([P, D], f8, tag="sq")
                nc.scalar.activation(
                    sq[:], po[:], mybir.ActivationFunctionType.Square,
                    accum_out=red_all[:, e : e + 1],
                )

            nc.sync.dma_start(loss[:], red_all[:])

    nc.finalize()
    return nc


def get_nc(act="gelu"):
    global _NC
    if _NC is None:
        _NC = _build_nc(act)
    return _NC


def make_in_maps(features, target_features, W1, b1, W2, b2):
    features = np.asarray(features, np.float32)
    target_features = np.asarray(target_features, np.float32)
    W1 = np.asarray(W1, np.float32)
    b1 = np.asarray(b1, np.float32)
    W2 = np.asarray(W2, np.float32)
    b2 = np.asarray(b2, np.float32)

    # pack weights partition-major: wpack[a][e][p, kc*H + col] = W1[a,e,kc*128+p,col]
    #                              wpack[a][e][p, 8192 + kc*D + d] = W2[a,e,kc*128+p,d]
    w1p = np.ascontiguousarray(
        W1.reshape(E, E, KC1, P, H).transpose(0, 1, 3, 2, 4).reshape(E, E, P, W1_COLS)
    ).astype(F8)
    w2p = np.ascontiguousarray(
        W2.reshape(E, E, MC, P, D).transpose(0, 1, 3, 2, 4).reshape(E, E, P, W2_COLS)
    ).astype(F8)
    wpk = np.concatenate([w1p, w2p], axis=3)  # [A, E, P, 16384] fp8

    in_maps = []
    for a in range(E):
        fa = features[:, a]  # [B, D]
        ftp = np.ascontiguousarray(fa.T.reshape(KC1, P, B).transpose(1, 0, 2)).astype(F8)
        tgtn = np.ascontiguousarray(-target_features[:, a]).astype(BF16)  # [B, D]
        b2pa = np.ascontiguousarray(b2[a][None]).astype(BF16)  # [1, E, D]
        b1pa = np.ascontiguousarray(b1[a].reshape(E, MC, P).transpose(2, 0, 1))  # [P,E,MC]
        in_maps.append(
            {"wpack": wpk[a], "ftp": ftp, "tgtn": tgtn, "b2p": b2pa,
             "b1p": b1pa}
        )
    return in_maps


def kernel(features, target_features, W1, b1, W2, b2):
    from concourse.bass_utils import run_bass_kernel_spmd

    nc = get_nc()
    in_maps = make_in_maps(features, target_features, W1, b1, W2, b2)
    res = run_bass_kernel_spmd(nc, in_maps, list(range(E)))
    total = sum(float(r["loss"].astype(np.float64).sum()) for r in res.results)
    return np.float32(total / (B * D * E))
